# revision 1
# baseline (speedup 1.0000x reference)
"""Trainium2 Bass kernel for nn_LocalInferenceModeling (cross-attention enhance).

Reference computation (per batch b):
    e = x1 @ x2^T                                  [L, L]
    a12 = softmax_j(e + m2[j]);  x1t = a12 @ x2    [L, H]
    a21 = softmax_i(e^T + m1[i]); x2t = a21 @ x1   [L, H]
    y1 = concat([x1, x1t, x1 - x1t, x1 * x1t], -1) [L, 4H]
    y2 = concat([x2, x2t, x2 - x2t, x2 * x2t], -1)

Sharding: batch dim B=32 split across 8 NeuronCores (4 batches/core),
no communication.  Masks (0 / -1e30 rows from seq_lengths) are computed
host-side and passed as extra inputs.

Per-core dataflow (per batch):
  - load x1, x2 natural [4x(128,1024)]
  - PE-transpose -> x1T, x2T [8x(128,512)]  (h on partitions)
  - e   [i,j]: matmul(lhsT=x1T, rhs=x2T) accum over 8 h-tiles (+ rank-1
    mask row via ones^T @ m2row matmul into the same PSUM bank)
  - softmax over free dim: reduce_max(negate) -> Exp(bias=-max) -> sum ->
    reciprocal (probs kept UNNORMALIZED; 1/z applied after stage-2)
  - e^T [j,i]: same with operands swapped, mask m1
  - PE-transpose probs p12 -> p12T (j on partitions)
  - x1t = p12T^T @ x2  (accum over 4 j-tiles, two N=512 halves)
  - normalize via activation(Copy, scale=1/z) and fuse enhance
    (sub/mul on DVE) into one [128, 3072] output tile -> DMA
  - x_bar copy slice of output DMA'd straight from the resident input tile
"""

import os
import sys

import numpy as np

sys.path.insert(0, "/opt/trn_rl_repo")

from contextlib import ExitStack

import concourse.bass as bass
import concourse.bacc as bacc
import concourse.mybir as mybir
from concourse import masks
from concourse.bass_utils import run_bass_kernel_spmd
from concourse.tile import TileContext

B, L, H = 32, 512, 1024
NCORES = 8
BPC = B // NCORES  # batches per core
NEG = np.float32(-1.0e30)

F32 = mybir.dt.float32
F32R = mybir.dt.float32r

# fp32r runs the PE at 1 cycle/row (vs 4 for fp32).  Accuracy is checked in
# test.py against the fp32 reference; flip these to F32 if it ever fails.
LOGIT_DT = F32R  # e / e^T matmuls
AV_DT = F32R  # probs @ values matmuls

NT = L // 128  # 4 partition tiles per L
HT = H // 128  # 8 partition tiles per H
Exp = mybir.ActivationFunctionType.Exp
Copy = mybir.ActivationFunctionType.Copy
AX = mybir.AxisListType.X

_NC_CACHE = {}


def _mm(ap, dt):
    return ap.bitcast(dt) if dt != F32 else ap


def build_nc():
    nc = bacc.Bacc(None, target_bir_lowering=False)
    x1 = nc.dram_tensor("x1", [BPC, L, H], F32, kind="ExternalInput")
    x2 = nc.dram_tensor("x2", [BPC, L, H], F32, kind="ExternalInput")
    m1 = nc.dram_tensor("m1", [BPC, L], F32, kind="ExternalInput")
    m2 = nc.dram_tensor("m2", [BPC, L], F32, kind="ExternalInput")
    y1 = nc.dram_tensor("y1", [BPC, L, 4 * H], F32, kind="ExternalOutput")
    y2 = nc.dram_tensor("y2", [BPC, L, 4 * H], F32, kind="ExternalOutput")

    with TileContext(nc) as tc, ExitStack() as ctx:
        from concourse.tile import add_dep_helper

        const = ctx.enter_context(tc.tile_pool(name="const", bufs=1))
        ident = const.tile([128, 128], F32)
        masks.make_identity(nc, ident[:])
        ones = const.tile([1, 128], F32)
        nc.vector.memset(ones[:], 1.0)

        xp = ctx.enter_context(tc.tile_pool(name="xp", bufs=NT + 2))
        xtp = ctx.enter_context(tc.tile_pool(name="xtp", bufs=HT))
        xrp = ctx.enter_context(tc.tile_pool(name="xrp", bufs=NT))
        pp = ctx.enter_context(tc.tile_pool(name="pp", bufs=NT))
        ptp = ctx.enter_context(tc.tile_pool(name="ptp", bufs=NT))
        st = ctx.enter_context(tc.tile_pool(name="st", bufs=4 * NT))
        yp = ctx.enter_context(tc.tile_pool(name="yp", bufs=3))
        mrp = ctx.enter_context(tc.tile_pool(name="mrp", bufs=1))
        esb = ctx.enter_context(tc.tile_pool(name="esb", bufs=2))
        psE = ctx.enter_context(tc.tile_pool(name="psE", bufs=2, space="PSUM"))
        psTX = ctx.enter_context(tc.tile_pool(name="psTX", bufs=2, space="PSUM"))
        psTP = ctx.enter_context(tc.tile_pool(name="psTP", bufs=2, space="PSUM"))
        psB = ctx.enter_context(tc.tile_pool(name="psB", bufs=1, space="PSUM"))
        psS = ctx.enter_context(tc.tile_pool(name="psS", bufs=1, space="PSUM"))
        scratch = psS.tile([32, 32], F32, name="scratch", tag="scratch")

        # Per-psum-tag history of "release touches": slot_gate[tag][k] is the
        # PE touch that observed the copy releasing that tag's k-th tile.
        gates = {"psE": [], "psTX": [], "psTP": [], "psB": []}

        def touch(ap):
            # Tiny PE transpose reading `ap` so the PE engine observes the
            # producer's sem tick; real matmuls then carry at most one sync
            # wait (walrus can encode only one on self-loading matmuls).
            a32 = ap[0:32, 0:32]
            if a32.dtype != F32:
                a32 = a32.bitcast(F32)
            with tc.high_priority(offset=200):
                return nc.tensor.transpose(scratch[:], a32, ident[0:32, 0:32])

        def gate(tag, bufs, first_inst):
            # Order the group's first PE write after the touch that observed
            # the release of the slot it reuses (bufs groups back).
            hist = gates[tag]
            k = len(hist)
            if k >= bufs and hist[k - bufs] is not None:
                add_dep_helper(first_inst.ins, hist[k - bufs].ins, sync=False,
                               reason="psum slot gate")
            hist.append(None)  # placeholder until release touch known
            return k

        def set_gate(tag, k, tinst):
            gates[tag][k] = tinst

        touch(ident)
        nc.tensor.matmul(scratch[0:32, 0:1], ones[0:1, 0:32], ones[0:1, 0:1],
                         start=True, stop=True)

        m1all = mrp.tile([1, BPC * L], F32, name="m1all", tag="m1all")
        m2all = mrp.tile([1, BPC * L], F32, name="m2all", tag="m2all")
        nc.sync.dma_start(m1all[:1, :], m1.rearrange("b l -> (b l)")[None, :])
        nc.sync.dma_start(m2all[:1, :], m2.rearrange("b l -> (b l)")[None, :])

        for b in range(BPC):
            # ---- load inputs (natural layout, l on partitions) ----
            xn1 = [xp.tile([128, H], F32, name="xn1", tag="xn1") for _ in range(NT)]
            xn2 = [xp.tile([128, H], F32, name="xn2", tag="xn2") for _ in range(NT)]
            for a in range(NT):
                nc.vector.memset(xn1[a][0:1, H - 1 : H], 0.0)
                nc.sync.dma_start(xn1[a][:], x1[b, 128 * a : 128 * (a + 1), :])
                nc.vector.memset(xn2[a][0:1, H - 1 : H], 0.0)
                nc.sync.dma_start(xn2[a][:], x2[b, 128 * a : 128 * (a + 1), :])
            m1row = m1all[:, L * b : L * (b + 1)]
            m2row = m2all[:, L * b : L * (b + 1)]
            xn_touch = [touch(t) for t in xn1 + xn2]
            if b == 0:
                nc.tensor.matmul(scratch[0:32, 0:1], m1row[0:1, 0:32],
                                 ones[0:1, 0:1], start=True, stop=True)
                nc.tensor.matmul(scratch[0:32, 0:1], m2row[0:1, 0:32],
                                 ones[0:1, 0:1], start=True, stop=True)

            # ---- fp32r shadows of natural x for the stage-2 matmuls ----
            x1T = [xtp.tile([128, L], F32R, name="x1T", tag="x1T") for _ in range(HT)]
            x2T = [xtp.tile([128, L], F32R, name="x2T", tag="x2T") for _ in range(HT)]
            x1r = [xrp.tile([128, H], F32R, name="x1r", tag="x1r") for _ in range(NT)]
            x2r = [xrp.tile([128, H], F32R, name="x2r", tag="x2r") for _ in range(NT)]
            for a in range(NT):
                nc.scalar.copy(x1r[a][:], xn1[a][:])
                touch(x1r[a])
                nc.scalar.copy(x2r[a][:], xn2[a][:])
                touch(x2r[a])

            # ---- transpose x -> xT (h on partitions) ----
            for srcn, dstT in ((xn1, x1T), (xn2, x2T)):
                for c in range(HT):
                    tt = psTX.tile([128, L], F32, name="psTX", tag="psTX")
                    k = None
                    for a in range(NT):
                        inst = nc.tensor.transpose(
                            tt[:, 128 * a : 128 * (a + 1)],
                            srcn[a][:, 128 * c : 128 * (c + 1)],
                            ident[:],
                        )
                        if a == 0:
                            k = gate("psTX", 2, inst)
                            add_dep_helper(inst.ins, xn_touch[-1].ins,
                                           sync=False, reason="xn touch gate")
                    nc.vector.tensor_copy(dstT[c][:], tt[:])
                    set_gate("psTX", k, touch(dstT[c]))

            # ---- logits + masked softmax stats (both orientations) ----
            p12 = [pp.tile([128, L], F32, name="p12", tag="p12") for _ in range(NT)]
            p21 = [pp.tile([128, L], F32, name="p21", tag="p21") for _ in range(NT)]
            rz1 = [st.tile([128, 1], F32, name="rz1", tag="rz1") for _ in range(NT)]
            rz2 = [st.tile([128, 1], F32, name="rz2", tag="rz2") for _ in range(NT)]
            for lhsT, rhsT, mrow, probs, rzs in (
                (x1T, x2T, m2row, p12, rz1),
                (x2T, x1T, m1row, p21, rz2),
            ):
                for a in range(NT):
                    pe = psE.tile([128, L], F32, name="psE", tag="psE")
                    k = None
                    for c in range(HT):
                        inst = nc.tensor.matmul(
                            pe[:],
                            lhsT[c][:, 128 * a : 128 * (a + 1)],
                            rhsT[c][:],
                            start=(c == 0),
                            stop=False,
                        )
                        if c == 0:
                            k = gate("psE", 2, inst)
                    # rank-1 broadcast of the mask row: ones^T @ mrow
                    nc.tensor.matmul(
                        pe[:], ones[:1, :], mrow[:1, :], start=False, stop=True
                    )
                    e_sb = esb.tile([128, L], F32, name="e_sb", tag="e_sb")
                    nc.vector.tensor_copy(e_sb[:], pe[:])
                    set_gate("psE", k, touch(e_sb))
                    negmax = st.tile([128, 1], F32, name="negmax", tag="negmax")
                    nc.vector.reduce_max(negmax[:], e_sb[:], axis=AX, negate=True)
                    nc.scalar.activation(probs[a][:], e_sb[:], Exp, bias=negmax[:])
                    touch(probs[a])
                    z = st.tile([128, 1], F32, name="z", tag="z")
                    nc.vector.reduce_sum(z[:], probs[a][:], axis=AX)
                    nc.vector.reciprocal(rzs[a][:], z[:])

            # ---- transpose probs (contraction dim onto partitions) ----
            p12T = [ptp.tile([128, L], F32R, name="p12T", tag="p12T") for _ in range(NT)]
            p21T = [ptp.tile([128, L], F32R, name="p21T", tag="p21T") for _ in range(NT)]
            for srcp, dstT in ((p12, p12T), (p21, p21T)):
                for c in range(NT):
                    tt = psTP.tile([128, L], F32, name="psTP", tag="psTP")
                    k = None
                    for a in range(NT):
                        inst = nc.tensor.transpose(
                            tt[:, 128 * a : 128 * (a + 1)],
                            srcp[a][:, 128 * c : 128 * (c + 1)],
                            ident[:],
                        )
                        if a == 0:
                            k = gate("psTP", 2, inst)
                    nc.scalar.copy(dstT[c][:], tt[:])
                    set_gate("psTP", k, touch(dstT[c]))

            # ---- stage 2: tilde = probs @ values, normalize, enhance ----
            for pT, vals, xnat, xsrc, rzs, y in (
                (p12T, x2r, xn1, x1, rz1, y1),
                (p21T, x1r, xn2, x2, rz2, y2),
            ):
                for a in range(NT):
                    ys = yp.tile([128, 3 * H], F32, name="ys", tag="ys")
                    nc.vector.memset(ys[0:1, 0:1], 0.0)
                    for n in range(2):
                        pt = psB.tile([128, 512], F32, name="psB", tag="psB")
                        k = None
                        for c in range(NT):
                            inst = nc.tensor.matmul(
                                pt[:],
                                pT[c][:, 128 * a : 128 * (a + 1)],
                                vals[c][:, 512 * n : 512 * (n + 1)],
                                start=(c == 0),
                                stop=(c == NT - 1),
                            )
                            if c == 0:
                                k = gate("psB", 1, inst)
                        nc.vector.tensor_scalar_mul(
                            ys[:, 512 * n : 512 * (n + 1)], pt[:], rzs[a][:]
                        )
                        set_gate("psB", k, touch(ys[:, 512 * n : 512 * (n + 1)]))
                    nc.vector.tensor_sub(ys[:, H : 2 * H], xnat[a][:], ys[:, 0:H])
                    nc.vector.tensor_mul(ys[:, 2 * H : 3 * H], xnat[a][:], ys[:, 0:H])
                    rows = slice(128 * a, 128 * (a + 1))
                    nc.sync.dma_start(y[b, rows, H : 4 * H], ys[:])
                    # x_bar slice: DRAM->DRAM, no SBUF dependency
                    nc.sync.dma_start(y[b, rows, 0:H], xsrc[b, rows, :])
    if not nc.is_finalized():
        nc.finalize()
    return nc


def kernel(x1_bar, seq_lengths1, x2_bar, seq_lengths2):
    x1_bar = np.ascontiguousarray(x1_bar, dtype=np.float32)
    x2_bar = np.ascontiguousarray(x2_bar, dtype=np.float32)
    ar = np.arange(L, dtype=np.int32)
    m1 = np.where(ar[None, :] >= np.asarray(seq_lengths1)[:, None], NEG, 0.0)
    m2 = np.where(ar[None, :] >= np.asarray(seq_lengths2)[:, None], NEG, 0.0)
    m1 = m1.astype(np.float32)
    m2 = m2.astype(np.float32)

    if "nc" not in _NC_CACHE:
        _NC_CACHE["nc"] = build_nc()
    nc = _NC_CACHE["nc"]

    in_maps = []
    for c in range(NCORES):
        s = slice(c * BPC, (c + 1) * BPC)
        in_maps.append({"x1": x1_bar[s], "x2": x2_bar[s], "m1": m1[s], "m2": m2[s]})

    res = run_bass_kernel_spmd(nc, in_maps, core_ids=list(range(NCORES)))
    y1 = np.concatenate([r["y1"] for r in res.results], axis=0)
    y2 = np.concatenate([r["y2"] for r in res.results], axis=0)
    return y1, y2



# revision 50
# speedup vs baseline: 2.5726x; 2.5726x over previous
"""Trainium2 Bass kernel for nn_LocalInferenceModeling (cross-attention enhance).

Reference computation (per batch b):
    e = x1 @ x2^T                                  [L, L]
    a12 = softmax_j(e + m2[j]);  x1t = a12 @ x2    [L, H]
    a21 = softmax_i(e^T + m1[i]); x2t = a21 @ x1   [L, H]
    y1 = concat([x1, x1t, x1 - x1t, x1 * x1t], -1) [L, 4H]
    y2 = concat([x2, x2t, x2 - x2t, x2 * x2t], -1)

Sharding: batch dim B=32 split across 8 NeuronCores (4 batches/core),
no communication.

Device-side redesign vs the fp32 baseline:
  - Host supplies bf16 inputs, both natural ([L,H]) and pre-transposed
    ([H,L]); PE matmuls run bf16 (1 cyc/row), halving DMA bytes.
  - e is computed ONCE; e^T comes from 16 exact fp32 PE transposes of
    the e SBUF copy instead of a second 32-matmul pass.
  - Probs are produced directly in TRANSPOSED (contraction-ready)
    layout, so the baseline's 32 per-batch probs transposes vanish:
      p12T[j,i] = exp(e^T[j,i] - rowmax_i)   (pad-j rows self-masked)
      p21T[i,j] = exp(e[i,j] - colmax_j + m1col[i])
    Masking uses a bf16-exact sentinel (-29952) for m2/m1 so that pad
    rows stay recoverable (the sentinel shift cancels against the
    matching shift in the subtracted stabilizer); true -1e30 masking
    enters only via per-partition activation bias where it is exact.
    Stabilizer (max) values are applied via rank-1 ones (x) row matmuls;
    their bf16 rounding is uniform per output row/col and cancels in the
    z-normalization.
  - z = sum(exp) comes from tiny N=1 matmuls against a ones column
    (partition-dim sums), normalization is folded into the psum->SBUF
    copy on the Activation engine, enhance (sub/mul) runs all-bf16 on
    DVE at 2x, outputs are written bf16 (3H slice only); the host
    upcasts and prepends the x_bar slice.
  - DMAs are spread over the three legal issue queues (SP / Activation /
    GpSimd) since queue occupancy, not bus bytes, is the limiter.
"""

import sys

import numpy as np

sys.path.insert(0, "/opt/trn_rl_repo")

from contextlib import ExitStack

import ml_dtypes

import concourse.bass as bass
import concourse.bacc as bacc
import concourse.bass_isa as bass_isa
import concourse.mybir as mybir
from concourse import masks
from concourse.bass_utils import run_bass_kernel_spmd
from concourse.tile import TileContext

B, L, H = 32, 512, 1024
NCORES = 8
BPC = B // NCORES  # batches per core
NT = L // 128  # 4 partition tiles per L
HT = H // 128  # 8 partition tiles per H

SENT = np.float32(29952.0)  # bf16-exact sentinel magnitude
NEG = np.float32(-1.0e30)

F32 = mybir.dt.float32
F32R = mybir.dt.float32r
BF16 = mybir.dt.bfloat16
NPBF16 = np.dtype(ml_dtypes.bfloat16)

Exp = mybir.ActivationFunctionType.Exp
Copy = mybir.ActivationFunctionType.Copy
AX = mybir.AxisListType.X

_NC_CACHE = {}


def build_nc():
    nc = bacc.Bacc(None, target_bir_lowering=False)
    xb1 = nc.dram_tensor("xb1", [BPC, L, H], BF16, kind="ExternalInput")
    xb2 = nc.dram_tensor("xb2", [BPC, L, H], BF16, kind="ExternalInput")
    xt1 = nc.dram_tensor("xt1", [BPC, H, L], F32R, kind="ExternalInput")
    xt2 = nc.dram_tensor("xt2", [BPC, H, L], F32R, kind="ExternalInput")
    m2row = nc.dram_tensor("m2row", [BPC, L], BF16, kind="ExternalInput")
    m2rowneg = nc.dram_tensor("m2rowneg", [BPC, L], BF16, kind="ExternalInput")
    m1rowneg = nc.dram_tensor("m1rowneg", [BPC, L], BF16, kind="ExternalInput")
    # partition-dim (column) masks, f32, pre-swizzled [128, BPC*NT]
    m1col = nc.dram_tensor("m1col", [128, BPC * NT], F32, kind="ExternalInput")
    m1colsent = nc.dram_tensor("m1colsent", [128, BPC * NT], F32, kind="ExternalInput")
    y1 = nc.dram_tensor("y1", [BPC, L, 3 * H], BF16, kind="ExternalOutput")
    y2 = nc.dram_tensor("y2", [BPC, L, 3 * H], BF16, kind="ExternalOutput")

    # DMA issue queues, round-robined
    dmaqs = [nc.sync, nc.scalar, nc.gpsimd]

    with TileContext(nc) as tc, ExitStack() as ctx:
        from concourse.tile import add_dep_helper

        const = ctx.enter_context(tc.tile_pool(name="const", bufs=1))
        ident = const.tile([128, 128], F32)
        masks.make_identity(nc, ident[:])
        onesb = const.tile([1, 128], BF16)
        nc.vector.memset(onesb[:], 1.0)
        onescol = const.tile([128, 1], BF16)
        nc.vector.memset(onescol[:], 1.0)
        ones32 = const.tile([1, 32], F32)
        nc.vector.memset(ones32[:], 1.0)
        ones32col = const.tile([128, 1], F32)
        nc.vector.memset(ones32col[:], 1.0)


        xp = ctx.enter_context(tc.tile_pool(name="xp", bufs=2))
        esb = ctx.enter_context(tc.tile_pool(name="esb", bufs=6))
        pp = ctx.enter_context(tc.tile_pool(name="pp", bufs=2 * NT))
        st = ctx.enter_context(tc.tile_pool(name="st", bufs=3))
        yp = ctx.enter_context(tc.tile_pool(name="yp", bufs=4))
        mrp = ctx.enter_context(tc.tile_pool(name="mrp", bufs=1))
        pmp = ctx.enter_context(tc.tile_pool(name="pmp", bufs=2))
        psE = ctx.enter_context(tc.tile_pool(name="psE", bufs=2, space="PSUM"))
        psT = ctx.enter_context(tc.tile_pool(name="psT", bufs=2, space="PSUM"))
        psB = ctx.enter_context(tc.tile_pool(name="psB", bufs=2, space="PSUM"))
        psS = ctx.enter_context(tc.tile_pool(name="psS", bufs=1, space="PSUM"))
        psScr = ctx.enter_context(tc.tile_pool(name="psScr", bufs=1, space="PSUM"))
        scratch = psScr.tile([32, 32], F32, name="scratch", tag="scratch")

        gates = {"psE": [], "psT": [], "psB": [], "psS": []}

        def touch(ap):
            # Tiny PE matmul reading `ap` so the PE engine observes the
            # producer's sem tick; real matmuls then carry at most one sync
            # wait.
            p = min(ap.shape[0], 32)
            f = min(ap.shape[1], 32)
            if ap.dtype == F32R:
                ap = ap.bitcast(F32)
            oc = onescol if ap.dtype == BF16 else ones32col
            with tc.high_priority(offset=200):
                return nc.tensor.matmul(
                    scratch[0:f, 0:1], ap[0:p, 0:f], oc[0:p, 0:1],
                    start=True, stop=True)

        def gate(tag, bufs, first_inst):
            # Order the group's first PE write after the touch that observed
            # the release of the slot it reuses (bufs groups back).
            hist = gates[tag]
            k = len(hist)
            if k >= bufs and hist[k - bufs] is not None:
                add_dep_helper(first_inst.ins, hist[k - bufs].ins, sync=False,
                               reason="psum slot gate")
            hist.append(None)
            return k

        def set_gate(tag, k, tinst):
            gates[tag][k] = tinst

        touch(ident)
        nc.tensor.matmul(scratch[0:32, 0:1], ones32[0:1, :], ones32[0:1, 0:1],
                         start=True, stop=True)

        # ---- static mask loads ----
        m2r = mrp.tile([1, BPC * L], BF16, name="m2r", tag="m2r")
        m2rn = mrp.tile([1, BPC * L], BF16, name="m2rn", tag="m2rn")
        m1rn = mrp.tile([1, BPC * L], BF16, name="m1rn", tag="m1rn")
        m1c = mrp.tile([128, BPC * NT], F32, name="m1c", tag="m1c")
        m1cs = mrp.tile([128, BPC * NT], F32, name="m1cs", tag="m1cs")
        nc.scalar.dma_start(m2r[:1, :], m2row.rearrange("b l -> (b l)")[None, :])
        nc.scalar.dma_start(m2rn[:1, :], m2rowneg.rearrange("b l -> (b l)")[None, :])
        nc.scalar.dma_start(m1rn[:1, :], m1rowneg.rearrange("b l -> (b l)")[None, :])
        nc.scalar.dma_start(m1c[:], m1col[:, :])
        nc.scalar.dma_start(m1cs[:], m1colsent[:, :])
        # no touches for the mask rows: each rank-1 matmul consuming them has
        # a single unobserved producer, which its own sem wait covers

        def load_batch(b):
            xb1t = xp.tile([128, NT * H], BF16, name="xb1t", tag="xb1t")
            xb2t = xp.tile([128, NT * H], BF16, name="xb2t", tag="xb2t")
            xt1t = xp.tile([128, HT * L], F32R, name="xt1t", tag="xt1t")
            xt2t = xp.tile([128, HT * L], F32R, name="xt2t", tag="xt2t")
            # transposed operands first: the e matmuls only need these
            dmaqs[0].dma_start(
                xt1t[:].rearrange("p (c l) -> p c l", c=HT),
                xt1[b].rearrange("(c p) l -> p c l", p=128))
            dmaqs[2].dma_start(
                xt2t[:].rearrange("p (c l) -> p c l", c=HT),
                xt2[b].rearrange("(c p) l -> p c l", p=128))
            dmaqs[0].dma_start(
                xb1t[:].rearrange("p (a h) -> p a h", a=NT),
                xb1[b].rearrange("(a p) h -> p a h", p=128))
            dmaqs[2].dma_start(
                xb2t[:].rearrange("p (a h) -> p a h", a=NT),
                xb2[b].rearrange("(a p) h -> p a h", p=128))
            return xt1t, xt2t, xb1t, xb2t

        nxt = load_batch(0)
        for b in range(BPC):
            xt1t, xt2t, xb1t, xb2t = nxt
            touch(xt1t)
            touch(xt2t)
            if b + 1 < BPC:
                nxt = load_batch(b + 1)

            m2row_b = m2r[0:1, L * b : L * (b + 1)]
            m2rowneg_b = m2rn[0:1, L * b : L * (b + 1)]
            m1rowneg_b = m1rn[0:1, L * b : L * (b + 1)]

            # ---- stage 1: e psum (raw + m2 sentinel), row stats, e_sb ----
            nm4 = st.tile([128, NT], F32, name="nm4", tag="nm4")
            e_sb = [esb.tile([128, L], F32, name="e_sb", tag="e_sb")
                    for _ in range(NT)]
            pm = [pmp.tile([128, L], F32, name="pm", tag="pm")
                  for _ in range(NT)]
            for a in range(NT):
                pe = psE.tile([128, L], F32, name="psE", tag="psE")
                k = None
                for c in range(HT):
                    inst = nc.tensor.matmul(
                        pe[:],
                        xt1t[:, L * c + 128 * a : L * c + 128 * (a + 1)],
                        xt2t[:, L * c : L * (c + 1)],
                        start=(c == 0),
                        stop=False,
                    )
                    if c == 0:
                        k = gate("psE", 2, inst)
                # m2 sentinel rank-1 (uniform -SENT on padded j columns)
                nc.tensor.matmul(pe[:], onesb[0:1, :], m2row_b,
                                 start=False, stop=True)
                # negmax over j (valid j exist; sentinel excludes padded j)
                nc.vector.reduce_max(nm4[:, a : a + 1], pe[:], axis=AX,
                                     negate=True)
                # e_sb = e + m2sent (+ m1 sentinel baked per-partition)
                nc.vector.tensor_scalar_add(
                    e_sb[a][:], pe[:],
                    m1cs[:, NT * b + a : NT * b + a + 1])
                set_gate("psE", k, touch(e_sb[a]))
                # per-chunk column max over i (m1 sentinel excludes masked i)
                nc.gpsimd.partition_all_reduce(
                    pm[a][:], e_sb[a][:], 128, bass_isa.ReduceOp.max)

            # nm4 -> row layout [1, 512] (per-column PE transposes, bf16 copy)
            nmps = psS.tile([1, L], F32, name="nmps", tag="psS")
            knm = None
            for a in range(NT):
                inst = nc.tensor.transpose(
                    nmps[0:1, 128 * a : 128 * (a + 1)], nm4[:, a : a + 1],
                    ident[:])
                if a == 0:
                    knm = gate("psS", 1, inst)
            nm1r = st.tile([1, L], BF16, name="nm1r", tag="nm1r")
            nc.vector.tensor_copy(nm1r[:], nmps[:])
            set_gate("psS", knm, touch(nm1r))

            # ---- e^T tiles: fp32 transpose + p12T ----
            p12T = [pp.tile([128, L], BF16, name="p12T", tag="p12T")
                    for _ in range(NT)]
            for c in range(NT):
                tt = psT.tile([128, L], F32, name="psT", tag="psT")
                k = None
                for a in range(NT):
                    # one accumulation group for the whole bank: the first
                    # transpose starts (marks the bank pending-zero), the
                    # rest overwrite their still-pending columns
                    inst = nc.tensor.matmul(
                        tt[:, 128 * a : 128 * (a + 1)],
                        e_sb[a][:, 128 * c : 128 * (c + 1)],
                        ident[:], is_transpose=True,
                        start=(a == 0), stop=False,
                    )
                    if a == 0:
                        k = gate("psT", 2, inst)
                # undo m1 sentinel on free i, then subtract rowmax_i
                nc.tensor.matmul(tt[:], onesb[0:1, :], m1rowneg_b,
                                 start=False, stop=False)
                nc.tensor.matmul(tt[:], onesb[0:1, :], nm1r[0:1, :],
                                 start=False, stop=True)
                nc.scalar.activation(p12T[c][:], tt[:], Exp)
                set_gate("psT", k, touch(p12T[c]))

            # combine the 4 partial column maxes, clean off the m2 sentinel
            # (keeps the value bf16-representable), negate -> ncmr row
            cm1 = st.tile([1, L], F32, name="cm1", tag="cm1")
            cm2 = st.tile([1, L], F32, name="cm2", tag="cm2")
            cm3 = st.tile([1, L], F32, name="cm3", tag="cm3")
            cm4 = st.tile([1, L], F32, name="cm4", tag="cm4")
            nc.vector.tensor_tensor(cm1[:], pm[0][0:1, :], pm[1][0:1, :],
                                    op=mybir.AluOpType.max)
            nc.vector.tensor_tensor(cm2[:], pm[2][0:1, :], pm[3][0:1, :],
                                    op=mybir.AluOpType.max)
            nc.vector.tensor_tensor(cm3[:], cm1[:], cm2[:],
                                    op=mybir.AluOpType.max)
            nc.vector.tensor_sub(cm4[:], cm3[:], m2row_b)
            ncmr = st.tile([1, L], BF16, name="ncmr", tag="ncmr")
            nc.vector.tensor_scalar_mul(ncmr[:], cm4[:], -1.0)
            touch(ncmr)

            # ---- p21T: restage e into psum, add stabilizer, exp ----
            p21T = [pp.tile([128, L], BF16, name="p21T", tag="p21T")
                    for _ in range(NT)]
            for a in range(NT):
                pb = psB.tile([128, L], F32, name="psB2", tag="psB")
                inst = nc.tensor.matmul(pb[:], onesb[0:1, :], ncmr[0:1, :],
                                        start=True, stop=False)
                k = gate("psB", 2, inst)
                # undo the m2 sentinel for padded-j columns
                nc.tensor.matmul(pb[:], onesb[0:1, :], m2rowneg_b,
                                 start=False, stop=True)
                # pb += e_sb (m1col sentinel rides along; the true -1e30
                # bias below dominates it on masked i rows)
                nc.vector.tensor_add(pb[:], pb[:], e_sb[a][:])
                nc.scalar.activation(
                    p21T[a][:], pb[:], Exp,
                    bias=m1c[:, NT * b + a : NT * b + a + 1])
                set_gate("psB", k, touch(p21T[a]))

            # ---- z sums via ones-column matmuls, reciprocals ----
            z1ps = psB.tile([128, NT], F32, name="z1ps", tag="psB")
            kz1 = None
            for a in range(NT):
                for c in range(NT):
                    inst = nc.tensor.matmul(
                        z1ps[:, a : a + 1],
                        p12T[c][:, 128 * a : 128 * (a + 1)],
                        onescol[:], start=(c == 0), stop=(c == NT - 1))
                    if a == 0 and c == 0:
                        kz1 = gate("psB", 2, inst)
            rz1 = st.tile([128, NT], F32, name="rz1", tag="rz1")
            nc.vector.reciprocal(rz1[:], z1ps[:])
            set_gate("psB", kz1, touch(rz1))

            z2ps = psB.tile([128, NT], F32, name="z2ps", tag="psB")
            kz2 = None
            for c in range(NT):
                for a in range(NT):
                    inst = nc.tensor.matmul(
                        z2ps[:, c : c + 1],
                        p21T[a][:, 128 * c : 128 * (c + 1)],
                        onescol[:], start=(a == 0), stop=(a == NT - 1))
                    if c == 0 and a == 0:
                        kz2 = gate("psB", 2, inst)
            rz2 = st.tile([128, NT], F32, name="rz2", tag="rz2")
            nc.vector.reciprocal(rz2[:], z2ps[:])
            set_gate("psB", kz2, touch(rz2))

            # stage-2 value operands (loaded early, only now needed by PE)
            touch(xb1t)
            touch(xb2t)

            # ---- stage 2 + enhance + output ----
            for oi, (pT, xval, xnat, rz, y) in enumerate((
                (p12T, xb2t, xb1t, rz1, y1),
                (p21T, xb1t, xb2t, rz2, y2),
            )):
                for a in range(NT):
                    ys = yp.tile([128, 3 * H], BF16, name="ys", tag="ys")
                    for n in range(2):
                        pt = psB.tile([128, 512], F32, name="psB", tag="psB")
                        k = None
                        for c in range(NT):
                            inst = nc.tensor.matmul(
                                pt[:],
                                pT[c][:, 128 * a : 128 * (a + 1)],
                                xval[:, H * c + 512 * n : H * c + 512 * (n + 1)],
                                start=(c == 0),
                                stop=(c == NT - 1),
                            )
                            if c == 0:
                                k = gate("psB", 2, inst)
                        nc.scalar.activation(
                            ys[:, 512 * n : 512 * (n + 1)], pt[:], Copy,
                            scale=rz[:, a : a + 1])
                        set_gate("psB", k, touch(ys[:, 512 * n : 512 * (n + 1)]))
                    xn = xnat[:, H * a : H * (a + 1)]
                    nc.vector.tensor_sub(ys[:, H : 2 * H], xn, ys[:, 0:H])
                    nc.vector.tensor_mul(ys[:, 2 * H : 3 * H], xn, ys[:, 0:H])
                    rows = slice(128 * a, 128 * (a + 1))
                    # outputs only on SP and Pool; the Act queue must stay
                    # free for Exp/normalize
                    qi = [0, 2, 0, 2, 2, 0, 2, 0][oi * NT + a]
                    dmaqs[qi].dma_start(y[b, rows, :], ys[:])
    if not nc.is_finalized():
        nc.finalize()
    return nc


def kernel(x1_bar, seq_lengths1, x2_bar, seq_lengths2):
    x1_bar = np.ascontiguousarray(x1_bar, dtype=np.float32)
    x2_bar = np.ascontiguousarray(x2_bar, dtype=np.float32)
    sl1 = np.asarray(seq_lengths1).astype(np.int32)
    sl2 = np.asarray(seq_lengths2).astype(np.int32)

    xb1 = x1_bar.astype(NPBF16)
    xb2 = x2_bar.astype(NPBF16)
    xt1 = np.ascontiguousarray(x1_bar.transpose(0, 2, 1))
    xt2 = np.ascontiguousarray(x2_bar.transpose(0, 2, 1))

    ar = np.arange(L, dtype=np.int32)
    pad1 = ar[None, :] >= sl1[:, None]  # [B, L] True on padded i
    pad2 = ar[None, :] >= sl2[:, None]
    m2row = np.where(pad2, -SENT, 0.0).astype(NPBF16)
    m2rowneg = np.where(pad2, SENT, 0.0).astype(NPBF16)
    m1rowneg = np.where(pad1, SENT, 0.0).astype(NPBF16)
    # col masks, swizzled to [128, B*NT]: col[p, b*NT+a] = mask[b, a*128+p]
    def swz(m, val):
        out = np.where(m, val, 0.0).astype(np.float32)  # [B, L]
        return np.ascontiguousarray(
            out.reshape(B, NT, 128).transpose(2, 0, 1).reshape(128, B * NT))
    m1col = swz(pad1, NEG)
    m1colsent = swz(pad1, -SENT)

    if "nc" not in _NC_CACHE:
        _NC_CACHE["nc"] = build_nc()
    nc = _NC_CACHE["nc"]

    in_maps = []
    for c in range(NCORES):
        s = slice(c * BPC, (c + 1) * BPC)
        sc = slice(c * BPC * NT, (c + 1) * BPC * NT)
        in_maps.append({
            "xb1": xb1[s], "xb2": xb2[s], "xt1": xt1[s], "xt2": xt2[s],
            "m2row": m2row[s], "m2rowneg": m2rowneg[s], "m1rowneg": m1rowneg[s],
            "m1col": m1col[:, sc], "m1colsent": m1colsent[:, sc],
        })

    res = run_bass_kernel_spmd(nc, in_maps, core_ids=list(range(NCORES)))
    yd1 = np.concatenate([r["y1"] for r in res.results], axis=0)
    yd2 = np.concatenate([r["y2"] for r in res.results], axis=0)

    y1 = np.empty((B, L, 4 * H), dtype=np.float32)
    y2 = np.empty((B, L, 4 * H), dtype=np.float32)
    y1[:, :, 0:H] = x1_bar
    y2[:, :, 0:H] = x2_bar
    y1[:, :, H:] = yd1.astype(np.float32)
    y2[:, :, H:] = yd2.astype(np.float32)
    return y1, y2


# revision 60
# speedup vs baseline: 2.6270x; 1.0212x over previous
"""Trainium2 Bass kernel for nn_LocalInferenceModeling (cross-attention enhance).

Reference computation (per batch b):
    e = x1 @ x2^T                                  [L, L]
    a12 = softmax_j(e + m2[j]);  x1t = a12 @ x2    [L, H]
    a21 = softmax_i(e^T + m1[i]); x2t = a21 @ x1   [L, H]
    y1 = concat([x1, x1t, x1 - x1t, x1 * x1t], -1) [L, 4H]
    y2 = concat([x2, x2t, x2 - x2t, x2 * x2t], -1)

Sharding: batch dim B=32 split across 8 NeuronCores (4 batches/core),
no communication.

Device-side redesign vs the fp32 baseline:
  - Host supplies bf16 inputs, both natural ([L,H]) and pre-transposed
    ([H,L]); PE matmuls run bf16 (1 cyc/row), halving DMA bytes.
  - e is computed ONCE; e^T comes from 16 exact fp32 PE transposes of
    the e SBUF copy instead of a second 32-matmul pass.
  - Probs are produced directly in TRANSPOSED (contraction-ready)
    layout, so the baseline's 32 per-batch probs transposes vanish:
      p12T[j,i] = exp(e^T[j,i] - rowmax_i)   (pad-j rows self-masked)
      p21T[i,j] = exp(e[i,j] - colmax_j + m1col[i])
    Masking uses a bf16-exact sentinel (-29952) for m2/m1 so that pad
    rows stay recoverable (the sentinel shift cancels against the
    matching shift in the subtracted stabilizer); true -1e30 masking
    enters only via per-partition activation bias where it is exact.
    Stabilizer (max) values are applied via rank-1 ones (x) row matmuls;
    their bf16 rounding is uniform per output row/col and cancels in the
    z-normalization.
  - z = sum(exp) comes from tiny N=1 matmuls against a ones column
    (partition-dim sums), normalization is folded into the psum->SBUF
    copy on the Activation engine, enhance (sub/mul) runs all-bf16 on
    DVE at 2x, outputs are written bf16 (3H slice only); the host
    upcasts and prepends the x_bar slice.
  - DMAs are spread over the three legal issue queues (SP / Activation /
    GpSimd) since queue occupancy, not bus bytes, is the limiter.
"""

import sys

import numpy as np

sys.path.insert(0, "/opt/trn_rl_repo")

from contextlib import ExitStack

import ml_dtypes

import concourse.bass as bass
import concourse.bacc as bacc
import concourse.bass_isa as bass_isa
import concourse.mybir as mybir
from concourse import masks
from concourse.bass_utils import run_bass_kernel_spmd
from concourse.tile import TileContext

B, L, H = 32, 512, 1024
NCORES = 8
BPC = B // NCORES  # batches per core
NT = L // 128  # 4 partition tiles per L
HT = H // 128  # 8 partition tiles per H

SENT = np.float32(29952.0)  # bf16-exact sentinel magnitude
NEG = np.float32(-1.0e30)

F32 = mybir.dt.float32
F32R = mybir.dt.float32r
BF16 = mybir.dt.bfloat16
NPBF16 = np.dtype(ml_dtypes.bfloat16)

Exp = mybir.ActivationFunctionType.Exp
Copy = mybir.ActivationFunctionType.Copy
AX = mybir.AxisListType.X

_NC_CACHE = {}


def build_nc():
    nc = bacc.Bacc(None, target_bir_lowering=False)
    xb1 = nc.dram_tensor("xb1", [BPC, L, H], BF16, kind="ExternalInput")
    xb2 = nc.dram_tensor("xb2", [BPC, L, H], BF16, kind="ExternalInput")
    xt1 = nc.dram_tensor("xt1", [BPC, H, L], F32R, kind="ExternalInput")
    xt2 = nc.dram_tensor("xt2", [BPC, H, L], F32R, kind="ExternalInput")
    m2row = nc.dram_tensor("m2row", [BPC, L], BF16, kind="ExternalInput")
    m2rowneg = nc.dram_tensor("m2rowneg", [BPC, L], BF16, kind="ExternalInput")
    m1rowneg = nc.dram_tensor("m1rowneg", [BPC, L], BF16, kind="ExternalInput")
    # partition-dim (column) masks, f32, pre-swizzled [128, BPC*NT]
    m1col = nc.dram_tensor("m1col", [128, BPC * NT], F32, kind="ExternalInput")
    m1colsent = nc.dram_tensor("m1colsent", [128, BPC * NT], F32, kind="ExternalInput")
    y1 = nc.dram_tensor("y1", [BPC, L, 3 * H], BF16, kind="ExternalOutput")
    y2 = nc.dram_tensor("y2", [BPC, L, 3 * H], BF16, kind="ExternalOutput")

    # DMA issue queues, round-robined
    dmaqs = [nc.sync, nc.scalar, nc.gpsimd]

    with TileContext(nc) as tc, ExitStack() as ctx:
        from concourse.tile import add_dep_helper

        const = ctx.enter_context(tc.tile_pool(name="const", bufs=1))
        ident = const.tile([128, 128], F32)
        masks.make_identity(nc, ident[:])
        onesb = const.tile([1, 128], BF16)
        nc.vector.memset(onesb[:], 1.0)
        onescol = const.tile([128, 1], BF16)
        nc.vector.memset(onescol[:], 1.0)
        ones32 = const.tile([1, 32], F32)
        nc.vector.memset(ones32[:], 1.0)
        ones32col = const.tile([128, 1], F32)
        nc.vector.memset(ones32col[:], 1.0)


        xp = ctx.enter_context(tc.tile_pool(name="xp", bufs=2))
        esb = ctx.enter_context(tc.tile_pool(name="esb", bufs=6))
        pp = ctx.enter_context(tc.tile_pool(name="pp", bufs=2 * NT))
        st = ctx.enter_context(tc.tile_pool(name="st", bufs=3))
        yp = ctx.enter_context(tc.tile_pool(name="yp", bufs=4))
        mrp = ctx.enter_context(tc.tile_pool(name="mrp", bufs=1))
        pmp = ctx.enter_context(tc.tile_pool(name="pmp", bufs=2))
        psE = ctx.enter_context(tc.tile_pool(name="psE", bufs=2, space="PSUM"))
        psT = ctx.enter_context(tc.tile_pool(name="psT", bufs=2, space="PSUM"))
        psB = ctx.enter_context(tc.tile_pool(name="psB", bufs=2, space="PSUM"))
        psS = ctx.enter_context(tc.tile_pool(name="psS", bufs=1, space="PSUM"))
        psScr = ctx.enter_context(tc.tile_pool(name="psScr", bufs=1, space="PSUM"))
        scratch = psScr.tile([32, 32], F32, name="scratch", tag="scratch")

        gates = {"psE": [], "psT": [], "psB": [], "psS": []}

        def touch(ap):
            # Tiny PE matmul reading `ap` so the PE engine observes the
            # producer's sem tick; real matmuls then carry at most one sync
            # wait.
            p = min(ap.shape[0], 32)
            f = min(ap.shape[1], 32)
            if ap.dtype == F32R:
                ap = ap.bitcast(F32)
            oc = onescol if ap.dtype == BF16 else ones32col
            with tc.high_priority(offset=200):
                return nc.tensor.matmul(
                    scratch[0:f, 0:1], ap[0:p, 0:f], oc[0:p, 0:1],
                    start=True, stop=True)

        def gate(tag, bufs, first_inst):
            # Order the group's first PE write after the touch that observed
            # the release of the slot it reuses (bufs groups back).
            hist = gates[tag]
            k = len(hist)
            if k >= bufs and hist[k - bufs] is not None:
                add_dep_helper(first_inst.ins, hist[k - bufs].ins, sync=False,
                               reason="psum slot gate")
            hist.append(None)
            return k

        def set_gate(tag, k, tinst):
            gates[tag][k] = tinst

        touch(ident)
        nc.tensor.matmul(scratch[0:32, 0:1], ones32[0:1, :], ones32[0:1, 0:1],
                         start=True, stop=True)

        # ---- static mask loads ----
        m2r = mrp.tile([1, BPC * L], BF16, name="m2r", tag="m2r")
        m2rn = mrp.tile([1, BPC * L], BF16, name="m2rn", tag="m2rn")
        m1rn = mrp.tile([1, BPC * L], BF16, name="m1rn", tag="m1rn")
        m1c = mrp.tile([128, BPC * NT], F32, name="m1c", tag="m1c")
        m1cs = mrp.tile([128, BPC * NT], F32, name="m1cs", tag="m1cs")
        nc.scalar.dma_start(m2r[:1, :], m2row.rearrange("b l -> (b l)")[None, :])
        nc.scalar.dma_start(m2rn[:1, :], m2rowneg.rearrange("b l -> (b l)")[None, :])
        nc.scalar.dma_start(m1rn[:1, :], m1rowneg.rearrange("b l -> (b l)")[None, :])
        nc.scalar.dma_start(m1c[:], m1col[:, :])
        nc.scalar.dma_start(m1cs[:], m1colsent[:, :])
        # no touches for the mask rows: each rank-1 matmul consuming them has
        # a single unobserved producer, which its own sem wait covers

        def load_batch(b):
            xb1t = xp.tile([128, NT * H], BF16, name="xb1t", tag="xb1t")
            xb2t = xp.tile([128, NT * H], BF16, name="xb2t", tag="xb2t")
            xt1t = xp.tile([128, HT * L], F32R, name="xt1t", tag="xt1t")
            xt2t = xp.tile([128, HT * L], F32R, name="xt2t", tag="xt2t")
            # transposed operands first: the e matmuls only need these
            dmaqs[0].dma_start(
                xt1t[:].rearrange("p (c l) -> p c l", c=HT),
                xt1[b].rearrange("(c p) l -> p c l", p=128))
            dmaqs[2].dma_start(
                xt2t[:].rearrange("p (c l) -> p c l", c=HT),
                xt2[b].rearrange("(c p) l -> p c l", p=128))
            dmaqs[0].dma_start(
                xb1t[:].rearrange("p (a h) -> p a h", a=NT),
                xb1[b].rearrange("(a p) h -> p a h", p=128))
            dmaqs[2].dma_start(
                xb2t[:].rearrange("p (a h) -> p a h", a=NT),
                xb2[b].rearrange("(a p) h -> p a h", p=128))
            return xt1t, xt2t, xb1t, xb2t

        nxt = load_batch(0)
        for b in range(BPC):
            xt1t, xt2t, xb1t, xb2t = nxt
            touch(xt1t)
            touch(xt2t)
            if b + 1 < BPC:
                nxt = load_batch(b + 1)

            m2row_b = m2r[0:1, L * b : L * (b + 1)]
            m2rowneg_b = m2rn[0:1, L * b : L * (b + 1)]
            m1rowneg_b = m1rn[0:1, L * b : L * (b + 1)]

            # ---- stage 1: e psum (raw + m2 sentinel), row stats, e_sb ----
            nm4 = st.tile([128, NT], F32, name="nm4", tag="nm4")
            e_sb = [esb.tile([128, L], F32, name="e_sb", tag="e_sb")
                    for _ in range(NT)]
            pm = [pmp.tile([128, L], F32, name="pm", tag="pm")
                  for _ in range(NT)]
            for a in range(NT):
                pe = psE.tile([128, L], F32, name="psE", tag="psE")
                k = None
                for c in range(HT):
                    inst = nc.tensor.matmul(
                        pe[:],
                        xt1t[:, L * c + 128 * a : L * c + 128 * (a + 1)],
                        xt2t[:, L * c : L * (c + 1)],
                        start=(c == 0),
                        stop=False,
                    )
                    if c == 0:
                        k = gate("psE", 2, inst)
                # m2 sentinel rank-1 (uniform -SENT on padded j columns)
                nc.tensor.matmul(pe[:], onesb[0:1, :], m2row_b,
                                 start=False, stop=True)
                # negmax over j (valid j exist; sentinel excludes padded j)
                nc.vector.reduce_max(nm4[:, a : a + 1], pe[:], axis=AX,
                                     negate=True)
                # e_sb = e + m2sent (+ m1 sentinel baked per-partition)
                nc.vector.tensor_scalar_add(
                    e_sb[a][:], pe[:],
                    m1cs[:, NT * b + a : NT * b + a + 1])
                set_gate("psE", k, touch(e_sb[a]))
                # per-chunk column max over i (m1 sentinel excludes masked i)
                nc.gpsimd.partition_all_reduce(
                    pm[a][:], e_sb[a][:], 128, bass_isa.ReduceOp.max)

            # nm4 -> row layout [1, 512] (per-column PE transposes, bf16 copy)
            nmps = psS.tile([1, L], F32, name="nmps", tag="psS")
            knm = None
            for a in range(NT):
                inst = nc.tensor.transpose(
                    nmps[0:1, 128 * a : 128 * (a + 1)], nm4[:, a : a + 1],
                    ident[:])
                if a == 0:
                    knm = gate("psS", 1, inst)
            nm1r = st.tile([1, L], BF16, name="nm1r", tag="nm1r")
            nc.vector.tensor_copy(nm1r[:], nmps[:])
            set_gate("psS", knm, touch(nm1r))

            # ---- e^T tiles: fp32 transpose + p12T (+ z1 partial sums) ----
            p12T = [pp.tile([128, L], BF16, name="p12T", tag="p12T")
                    for _ in range(NT)]
            z1ps = psS.tile([128, NT], F32, name="z1ps", tag="psS")
            kz1 = None
            for c in range(NT):
                tt = psT.tile([128, L], F32, name="psT", tag="psT")
                k = None
                for a in range(NT):
                    # one accumulation group for the whole bank: the first
                    # transpose starts (marks the bank pending-zero), the
                    # rest overwrite their still-pending columns
                    inst = nc.tensor.matmul(
                        tt[:, 128 * a : 128 * (a + 1)],
                        e_sb[a][:, 128 * c : 128 * (c + 1)],
                        ident[:], is_transpose=True,
                        start=(a == 0), stop=False,
                    )
                    if a == 0:
                        k = gate("psT", 2, inst)
                # undo m1 sentinel on free i, then subtract rowmax_i
                nc.tensor.matmul(tt[:], onesb[0:1, :], m1rowneg_b,
                                 start=False, stop=False)
                nc.tensor.matmul(tt[:], onesb[0:1, :], nm1r[0:1, :],
                                 start=False, stop=True)
                nc.scalar.activation(p12T[c][:], tt[:], Exp)
                set_gate("psT", k, touch(p12T[c]))
                for a in range(NT):
                    inst = nc.tensor.matmul(
                        z1ps[:, a : a + 1],
                        p12T[c][:, 128 * a : 128 * (a + 1)],
                        onescol[:], start=(c == 0 and a == 0),
                        stop=(c == NT - 1 and a == NT - 1))
                    if c == 0 and a == 0:
                        kz1 = gate("psS", 1, inst)

            # combine the 4 partial column maxes, clean off the m2 sentinel
            # (keeps the value bf16-representable), negate -> ncmr row
            cm1 = st.tile([1, L], F32, name="cm1", tag="cm1")
            cm2 = st.tile([1, L], F32, name="cm2", tag="cm2")
            cm3 = st.tile([1, L], F32, name="cm3", tag="cm3")
            cm4 = st.tile([1, L], F32, name="cm4", tag="cm4")
            nc.vector.tensor_tensor(cm1[:], pm[0][0:1, :], pm[1][0:1, :],
                                    op=mybir.AluOpType.max)
            nc.vector.tensor_tensor(cm2[:], pm[2][0:1, :], pm[3][0:1, :],
                                    op=mybir.AluOpType.max)
            nc.vector.tensor_tensor(cm3[:], cm1[:], cm2[:],
                                    op=mybir.AluOpType.max)
            nc.vector.tensor_sub(cm4[:], cm3[:], m2row_b)
            ncmr = st.tile([1, L], BF16, name="ncmr", tag="ncmr")
            nc.vector.tensor_scalar_mul(ncmr[:], cm4[:], -1.0)
            touch(ncmr)

            # ---- p21T: restage e into psum, add stabilizer, exp (+ z2) ----
            p21T = [pp.tile([128, L], BF16, name="p21T", tag="p21T")
                    for _ in range(NT)]
            z2ps = psS.tile([128, NT], F32, name="z2ps", tag="psS")
            kz2 = None
            for a in range(NT):
                pb = psB.tile([128, L], F32, name="psB2", tag="psB")
                inst = nc.tensor.matmul(pb[:], onesb[0:1, :], ncmr[0:1, :],
                                        start=True, stop=False)
                k = gate("psB", 2, inst)
                # undo the m2 sentinel for padded-j columns
                nc.tensor.matmul(pb[:], onesb[0:1, :], m2rowneg_b,
                                 start=False, stop=True)
                # pb += e_sb (m1col sentinel rides along; the true -1e30
                # bias below dominates it on masked i rows)
                nc.vector.tensor_add(pb[:], pb[:], e_sb[a][:])
                nc.scalar.activation(
                    p21T[a][:], pb[:], Exp,
                    bias=m1c[:, NT * b + a : NT * b + a + 1])
                set_gate("psB", k, touch(p21T[a]))
                for c in range(NT):
                    inst = nc.tensor.matmul(
                        z2ps[:, c : c + 1],
                        p21T[a][:, 128 * c : 128 * (c + 1)],
                        onescol[:], start=(a == 0 and c == 0),
                        stop=(a == NT - 1 and c == NT - 1))
                    if a == 0 and c == 0:
                        kz2 = gate("psS", 1, inst)

            rz1 = st.tile([128, NT], F32, name="rz1", tag="rz1")
            nc.vector.reciprocal(rz1[:], z1ps[:])
            set_gate("psS", kz1, touch(rz1))

            rz2 = st.tile([128, NT], F32, name="rz2", tag="rz2")
            nc.vector.reciprocal(rz2[:], z2ps[:])
            set_gate("psS", kz2, touch(rz2))

            # stage-2 value operands (loaded early, only now needed by PE)
            touch(xb1t)
            touch(xb2t)

            # ---- stage 2 + enhance + output ----
            for oi, (pT, xval, xnat, rz, y) in enumerate((
                (p12T, xb2t, xb1t, rz1, y1),
                (p21T, xb1t, xb2t, rz2, y2),
            )):
                for a in range(NT):
                    ys = yp.tile([128, 3 * H], BF16, name="ys", tag="ys")
                    for n in range(2):
                        pt = psB.tile([128, 512], F32, name="psB", tag="psB")
                        k = None
                        for c in range(NT):
                            inst = nc.tensor.matmul(
                                pt[:],
                                pT[c][:, 128 * a : 128 * (a + 1)],
                                xval[:, H * c + 512 * n : H * c + 512 * (n + 1)],
                                start=(c == 0),
                                stop=(c == NT - 1),
                            )
                            if c == 0:
                                k = gate("psB", 2, inst)
                        nc.scalar.activation(
                            ys[:, 512 * n : 512 * (n + 1)], pt[:], Copy,
                            scale=rz[:, a : a + 1])
                        set_gate("psB", k, touch(ys[:, 512 * n : 512 * (n + 1)]))
                    xn = xnat[:, H * a : H * (a + 1)]
                    nc.vector.tensor_sub(ys[:, H : 2 * H], xn, ys[:, 0:H])
                    nc.vector.tensor_mul(ys[:, 2 * H : 3 * H], xn, ys[:, 0:H])
                    rows = slice(128 * a, 128 * (a + 1))
                    if b == BPC - 1 and oi == 1 and a == NT - 1:
                        # last tile: split across all queues to cut the tail
                        for qq in range(3):
                            dmaqs[qq].dma_start(
                                y[b, rows, qq * H : (qq + 1) * H],
                                ys[:, qq * H : (qq + 1) * H])
                    else:
                        # outputs only on SP and Pool; the Act queue must
                        # stay free for Exp/normalize
                        qi = [0, 2, 0, 2, 2, 0, 2, 0][oi * NT + a]
                        dmaqs[qi].dma_start(y[b, rows, :], ys[:])
    if not nc.is_finalized():
        nc.finalize()
    return nc


def kernel(x1_bar, seq_lengths1, x2_bar, seq_lengths2):
    x1_bar = np.ascontiguousarray(x1_bar, dtype=np.float32)
    x2_bar = np.ascontiguousarray(x2_bar, dtype=np.float32)
    sl1 = np.asarray(seq_lengths1).astype(np.int32)
    sl2 = np.asarray(seq_lengths2).astype(np.int32)

    xb1 = x1_bar.astype(NPBF16)
    xb2 = x2_bar.astype(NPBF16)
    xt1 = np.ascontiguousarray(x1_bar.transpose(0, 2, 1))
    xt2 = np.ascontiguousarray(x2_bar.transpose(0, 2, 1))

    ar = np.arange(L, dtype=np.int32)
    pad1 = ar[None, :] >= sl1[:, None]  # [B, L] True on padded i
    pad2 = ar[None, :] >= sl2[:, None]
    m2row = np.where(pad2, -SENT, 0.0).astype(NPBF16)
    m2rowneg = np.where(pad2, SENT, 0.0).astype(NPBF16)
    m1rowneg = np.where(pad1, SENT, 0.0).astype(NPBF16)
    # col masks, swizzled to [128, B*NT]: col[p, b*NT+a] = mask[b, a*128+p]
    def swz(m, val):
        out = np.where(m, val, 0.0).astype(np.float32)  # [B, L]
        return np.ascontiguousarray(
            out.reshape(B, NT, 128).transpose(2, 0, 1).reshape(128, B * NT))
    m1col = swz(pad1, NEG)
    m1colsent = swz(pad1, -SENT)

    if "nc" not in _NC_CACHE:
        _NC_CACHE["nc"] = build_nc()
    nc = _NC_CACHE["nc"]

    in_maps = []
    for c in range(NCORES):
        s = slice(c * BPC, (c + 1) * BPC)
        sc = slice(c * BPC * NT, (c + 1) * BPC * NT)
        in_maps.append({
            "xb1": xb1[s], "xb2": xb2[s], "xt1": xt1[s], "xt2": xt2[s],
            "m2row": m2row[s], "m2rowneg": m2rowneg[s], "m1rowneg": m1rowneg[s],
            "m1col": m1col[:, sc], "m1colsent": m1colsent[:, sc],
        })

    res = run_bass_kernel_spmd(nc, in_maps, core_ids=list(range(NCORES)))
    yd1 = np.concatenate([r["y1"] for r in res.results], axis=0)
    yd2 = np.concatenate([r["y2"] for r in res.results], axis=0)

    y1 = np.empty((B, L, 4 * H), dtype=np.float32)
    y2 = np.empty((B, L, 4 * H), dtype=np.float32)
    y1[:, :, 0:H] = x1_bar
    y2[:, :, 0:H] = x2_bar
    y1[:, :, H:] = yd1.astype(np.float32)
    y2[:, :, H:] = yd2.astype(np.float32)
    return y1, y2


# revision 71
# speedup vs baseline: 2.7602x; 1.0507x over previous
"""Trainium2 Bass kernel for nn_LocalInferenceModeling (cross-attention enhance).

Reference computation (per batch b):
    e = x1 @ x2^T                                  [L, L]
    a12 = softmax_j(e + m2[j]);  x1t = a12 @ x2    [L, H]
    a21 = softmax_i(e^T + m1[i]); x2t = a21 @ x1   [L, H]
    y1 = concat([x1, x1t, x1 - x1t, x1 * x1t], -1) [L, 4H]
    y2 = concat([x2, x2t, x2 - x2t, x2 * x2t], -1)

Sharding: batch dim B=32 split across 8 NeuronCores (4 batches/core),
no communication.

Device-side redesign vs the fp32 baseline:
  - Host supplies bf16 inputs, both natural ([L,H]) and pre-transposed
    ([H,L]); PE matmuls run bf16 (1 cyc/row), halving DMA bytes.
  - e is computed ONCE; e^T comes from 16 exact fp32 PE transposes of
    the e SBUF copy instead of a second 32-matmul pass.
  - Probs are produced directly in TRANSPOSED (contraction-ready)
    layout, so the baseline's 32 per-batch probs transposes vanish:
      p12T[j,i] = exp(e^T[j,i] - rowmax_i)   (pad-j rows self-masked)
      p21T[i,j] = exp(e[i,j] - colmax_j + m1col[i])
    Masking uses a bf16-exact sentinel (-29952) for m2/m1 so that pad
    rows stay recoverable (the sentinel shift cancels against the
    matching shift in the subtracted stabilizer); true -1e30 masking
    enters only via per-partition activation bias where it is exact.
    Stabilizer (max) values are applied via rank-1 ones (x) row matmuls;
    their bf16 rounding is uniform per output row/col and cancels in the
    z-normalization.
  - z = sum(exp) comes from tiny N=1 matmuls against a ones column
    (partition-dim sums), normalization is folded into the psum->SBUF
    copy on the Activation engine, enhance (sub/mul) runs all-bf16 on
    DVE at 2x, outputs are written bf16 (3H slice only); the host
    upcasts and prepends the x_bar slice.
  - DMAs are spread over the three legal issue queues (SP / Activation /
    GpSimd) since queue occupancy, not bus bytes, is the limiter.
"""

import sys

import numpy as np

sys.path.insert(0, "/opt/trn_rl_repo")

from contextlib import ExitStack

import ml_dtypes

import concourse.bass as bass
import concourse.bacc as bacc
import concourse.bass_isa as bass_isa
import concourse.mybir as mybir
from concourse import masks
from concourse.bass_utils import run_bass_kernel_spmd
from concourse.tile import TileContext

B, L, H = 32, 512, 1024
NCORES = 8
BPC = B // NCORES  # batches per core
NT = L // 128  # 4 partition tiles per L
HT = H // 128  # 8 partition tiles per H

SENT = np.float32(29952.0)  # bf16-exact sentinel magnitude
NEG = np.float32(-1.0e30)

F32 = mybir.dt.float32
F32R = mybir.dt.float32r
BF16 = mybir.dt.bfloat16
NPBF16 = np.dtype(ml_dtypes.bfloat16)

Exp = mybir.ActivationFunctionType.Exp
Copy = mybir.ActivationFunctionType.Copy
AX = mybir.AxisListType.X

_NC_CACHE = {}


def build_nc():
    nc = bacc.Bacc(None, target_bir_lowering=False)
    xb1 = nc.dram_tensor("xb1", [BPC, L, H], BF16, kind="ExternalInput")
    xb2 = nc.dram_tensor("xb2", [BPC, L, H], BF16, kind="ExternalInput")
    xt1 = nc.dram_tensor("xt1", [BPC, H, L], F32R, kind="ExternalInput")
    xt2 = nc.dram_tensor("xt2", [BPC, H, L], F32R, kind="ExternalInput")
    m2row = nc.dram_tensor("m2row", [BPC, L], BF16, kind="ExternalInput")
    m2rowneg = nc.dram_tensor("m2rowneg", [BPC, L], BF16, kind="ExternalInput")
    m1rowneg = nc.dram_tensor("m1rowneg", [BPC, L], BF16, kind="ExternalInput")
    # partition-dim (column) masks, f32, pre-swizzled [128, BPC*NT]
    m1col = nc.dram_tensor("m1col", [128, BPC * NT], F32, kind="ExternalInput")
    m1colsent = nc.dram_tensor("m1colsent", [128, BPC * NT], F32, kind="ExternalInput")
    y1 = nc.dram_tensor("y1", [BPC, L, 3 * H], BF16, kind="ExternalOutput")
    y2 = nc.dram_tensor("y2", [BPC, L, 3 * H], BF16, kind="ExternalOutput")

    # DMA issue queues, round-robined
    dmaqs = [nc.sync, nc.scalar, nc.gpsimd]

    with TileContext(nc) as tc, ExitStack() as ctx:
        from concourse.tile import add_dep_helper

        const = ctx.enter_context(tc.tile_pool(name="const", bufs=1))
        ident = const.tile([128, 128], F32)
        masks.make_identity(nc, ident[:])
        onesb = const.tile([1, 128], BF16)
        nc.vector.memset(onesb[:], 1.0)
        onescol = const.tile([128, 1], BF16)
        nc.vector.memset(onescol[:], 1.0)
        ones32 = const.tile([1, 32], F32)
        nc.vector.memset(ones32[:], 1.0)
        ones32col = const.tile([128, 1], F32)
        nc.vector.memset(ones32col[:], 1.0)


        xp = ctx.enter_context(tc.tile_pool(name="xp", bufs=2))
        esb = ctx.enter_context(tc.tile_pool(name="esb", bufs=6))
        pp = ctx.enter_context(tc.tile_pool(name="pp", bufs=2 * NT))
        st = ctx.enter_context(tc.tile_pool(name="st", bufs=3))
        yp = ctx.enter_context(tc.tile_pool(name="yp", bufs=4))
        mrp = ctx.enter_context(tc.tile_pool(name="mrp", bufs=1))
        pmp = ctx.enter_context(tc.tile_pool(name="pmp", bufs=2))
        psE = ctx.enter_context(tc.tile_pool(name="psE", bufs=2, space="PSUM"))
        psT = ctx.enter_context(tc.tile_pool(name="psT", bufs=2, space="PSUM"))
        psB = ctx.enter_context(tc.tile_pool(name="psB", bufs=2, space="PSUM"))
        psS = ctx.enter_context(tc.tile_pool(name="psS", bufs=1, space="PSUM"))
        psScr = ctx.enter_context(tc.tile_pool(name="psScr", bufs=1, space="PSUM"))
        scratch = psScr.tile([32, 32], F32, name="scratch", tag="scratch")

        gates = {"psE": [], "psT": [], "psB": [], "psS": []}

        def touch(ap):
            # Tiny PE matmul reading `ap` so the PE engine observes the
            # producer's sem tick; real matmuls then carry at most one sync
            # wait.
            p = min(ap.shape[0], 32)
            f = min(ap.shape[1], 32)
            if ap.dtype == F32R:
                ap = ap.bitcast(F32)
            oc = onescol if ap.dtype == BF16 else ones32col
            with tc.high_priority(offset=200):
                return nc.tensor.matmul(
                    scratch[0:f, 0:1], ap[0:p, 0:f], oc[0:p, 0:1],
                    start=True, stop=True)

        def gate(tag, bufs, first_inst):
            # Order the group's first PE write after the touch that observed
            # the release of the slot it reuses (bufs groups back).
            hist = gates[tag]
            k = len(hist)
            if k >= bufs and hist[k - bufs] is not None:
                add_dep_helper(first_inst.ins, hist[k - bufs].ins, sync=False,
                               reason="psum slot gate")
            hist.append(None)
            return k

        def set_gate(tag, k, tinst):
            gates[tag][k] = tinst

        touch(ident)
        nc.tensor.matmul(scratch[0:32, 0:1], ones32[0:1, :], ones32[0:1, 0:1],
                         start=True, stop=True)

        # ---- static mask loads ----
        m2r = mrp.tile([1, BPC * L], BF16, name="m2r", tag="m2r")
        m2rn = mrp.tile([1, BPC * L], BF16, name="m2rn", tag="m2rn")
        m1rn = mrp.tile([1, BPC * L], BF16, name="m1rn", tag="m1rn")
        m1c = mrp.tile([128, BPC * NT], F32, name="m1c", tag="m1c")
        m1cs = mrp.tile([128, BPC * NT], F32, name="m1cs", tag="m1cs")
        # load order matters for batch 0: m1cs feeds the very first e_sb
        # adds, m2r the first rank-1; m2rn/m1rn are needed only later
        nc.scalar.dma_start(m1cs[:], m1colsent[:, :])
        nc.scalar.dma_start(m1c[:], m1col[:, :])
        nc.scalar.dma_start(m2r[:1, :], m2row.rearrange("b l -> (b l)")[None, :])
        nc.scalar.dma_start(m2rn[:1, :], m2rowneg.rearrange("b l -> (b l)")[None, :])
        nc.scalar.dma_start(m1rn[:1, :], m1rowneg.rearrange("b l -> (b l)")[None, :])
        # no touches for the mask rows: each rank-1 matmul consuming them has
        # a single unobserved producer, which its own sem wait covers

        def load_batch(b):
            xb1t = xp.tile([128, NT * H], BF16, name="xb1t", tag="xb1t")
            xb2t = xp.tile([128, NT * H], BF16, name="xb2t", tag="xb2t")
            xt1t = xp.tile([128, HT * L], F32R, name="xt1t", tag="xt1t")
            xt2t = xp.tile([128, HT * L], F32R, name="xt2t", tag="xt2t")
            # transposed operands first: the e matmuls only need these
            dmaqs[0].dma_start(
                xt1t[:].rearrange("p (c l) -> p c l", c=HT),
                xt1[b].rearrange("(c p) l -> p c l", p=128))
            dmaqs[2].dma_start(
                xt2t[:].rearrange("p (c l) -> p c l", c=HT),
                xt2[b].rearrange("(c p) l -> p c l", p=128))
            dmaqs[0].dma_start(
                xb1t[:].rearrange("p (a h) -> p a h", a=NT),
                xb1[b].rearrange("(a p) h -> p a h", p=128))
            dmaqs[2].dma_start(
                xb2t[:].rearrange("p (a h) -> p a h", a=NT),
                xb2[b].rearrange("(a p) h -> p a h", p=128))
            return xt1t, xt2t, xb1t, xb2t

        def emit_head(b, xt1t, xt2t):
            """e psum (raw + m2 sentinel), row stats, e_sb, nm1r."""
            touch(xt1t)
            touch(xt2t)
            m2row_b = m2r[0:1, L * b : L * (b + 1)]
            nm4 = st.tile([128, NT], F32, name="nm4", tag="nm4")
            e_sb = [esb.tile([128, L], F32, name="e_sb", tag="e_sb")
                    for _ in range(NT)]
            pm = [pmp.tile([128, L], F32, name="pm", tag="pm")
                  for _ in range(NT)]
            for a in range(NT):
                pe = psE.tile([128, L], F32, name="psE", tag="psE")
                k = None
                for c in range(HT):
                    inst = nc.tensor.matmul(
                        pe[:],
                        xt1t[:, L * c + 128 * a : L * c + 128 * (a + 1)],
                        xt2t[:, L * c : L * (c + 1)],
                        start=(c == 0),
                        stop=False,
                    )
                    if c == 0:
                        k = gate("psE", 2, inst)
                # m2 sentinel rank-1 (uniform -SENT on padded j columns)
                nc.tensor.matmul(pe[:], onesb[0:1, :], m2row_b,
                                 start=False, stop=True)
                # negmax over j (valid j exist; sentinel excludes padded j)
                nc.vector.reduce_max(nm4[:, a : a + 1], pe[:], axis=AX,
                                     negate=True)
                # e_sb = e + m2sent (+ m1 sentinel baked per-partition)
                nc.vector.tensor_scalar_add(
                    e_sb[a][:], pe[:],
                    m1cs[:, NT * b + a : NT * b + a + 1])
                set_gate("psE", k, touch(e_sb[a]))
                # per-chunk column max over i (m1 sentinel excludes masked i)
                nc.gpsimd.partition_all_reduce(
                    pm[a][:], e_sb[a][:], 128, bass_isa.ReduceOp.max)

            # nm4 -> row layout [1, 512] (per-column PE transposes, bf16 copy)
            nmps = psS.tile([1, L], F32, name="nmps", tag="psS")
            knm = None
            for a in range(NT):
                inst = nc.tensor.transpose(
                    nmps[0:1, 128 * a : 128 * (a + 1)], nm4[:, a : a + 1],
                    ident[:])
                if a == 0:
                    knm = gate("psS", 1, inst)
            nm1r = st.tile([1, L], BF16, name="nm1r", tag="nm1r")
            nc.vector.tensor_copy(nm1r[:], nmps[:])
            set_gate("psS", knm, touch(nm1r))
            return e_sb, pm, nm1r

        nxt = load_batch(0)
        heads = {}
        for b in range(BPC):
            xt1t, xt2t, xb1t, xb2t = nxt
            if b + 1 < BPC:
                nxt = load_batch(b + 1)

            m2row_b = m2r[0:1, L * b : L * (b + 1)]
            m2rowneg_b = m2rn[0:1, L * b : L * (b + 1)]
            m1rowneg_b = m1rn[0:1, L * b : L * (b + 1)]

            if b in heads:
                e_sb, pm, nm1r = heads.pop(b)
            else:
                e_sb, pm, nm1r = emit_head(b, xt1t, xt2t)

            # ---- e^T tiles: fp32 transpose + p12T (+ z1 partial sums) ----
            p12T = [pp.tile([128, L], BF16, name="p12T", tag="p12T")
                    for _ in range(NT)]
            z1ps = psS.tile([128, NT], F32, name="z1ps", tag="psS")
            kz1 = None
            for c in range(NT):
                tt = psT.tile([128, L], F32, name="psT", tag="psT")
                k = None
                for a in range(NT):
                    # one accumulation group for the whole bank: the first
                    # transpose starts (marks the bank pending-zero), the
                    # rest overwrite their still-pending columns
                    inst = nc.tensor.matmul(
                        tt[:, 128 * a : 128 * (a + 1)],
                        e_sb[a][:, 128 * c : 128 * (c + 1)],
                        ident[:], is_transpose=True,
                        start=(a == 0), stop=False,
                    )
                    if a == 0:
                        k = gate("psT", 2, inst)
                # undo m1 sentinel on free i, then subtract rowmax_i
                nc.tensor.matmul(tt[:], onesb[0:1, :], m1rowneg_b,
                                 start=False, stop=False)
                nc.tensor.matmul(tt[:], onesb[0:1, :], nm1r[0:1, :],
                                 start=False, stop=True)
                nc.scalar.activation(p12T[c][:], tt[:], Exp)
                set_gate("psT", k, touch(p12T[c]))
                for a in range(NT):
                    inst = nc.tensor.matmul(
                        z1ps[:, a : a + 1],
                        p12T[c][:, 128 * a : 128 * (a + 1)],
                        onescol[:], start=(c == 0 and a == 0),
                        stop=(c == NT - 1 and a == NT - 1))
                    if c == 0 and a == 0:
                        kz1 = gate("psS", 1, inst)

            # combine the 4 partial column maxes, clean off the m2 sentinel
            # (keeps the value bf16-representable), negate -> ncmr row
            cm1 = st.tile([1, L], F32, name="cm1", tag="cm1")
            cm2 = st.tile([1, L], F32, name="cm2", tag="cm2")
            cm3 = st.tile([1, L], F32, name="cm3", tag="cm3")
            cm4 = st.tile([1, L], F32, name="cm4", tag="cm4")
            nc.vector.tensor_tensor(cm1[:], pm[0][0:1, :], pm[1][0:1, :],
                                    op=mybir.AluOpType.max)
            nc.vector.tensor_tensor(cm2[:], pm[2][0:1, :], pm[3][0:1, :],
                                    op=mybir.AluOpType.max)
            nc.vector.tensor_tensor(cm3[:], cm1[:], cm2[:],
                                    op=mybir.AluOpType.max)
            nc.vector.tensor_sub(cm4[:], cm3[:], m2row_b)
            ncmr = st.tile([1, L], BF16, name="ncmr", tag="ncmr")
            nc.vector.tensor_scalar_mul(ncmr[:], cm4[:], -1.0)
            touch(ncmr)

            # ---- p21T: restage e into psum, add stabilizer, exp (+ z2) ----
            p21T = [pp.tile([128, L], BF16, name="p21T", tag="p21T")
                    for _ in range(NT)]
            z2ps = psS.tile([128, NT], F32, name="z2ps", tag="psS")
            kz2 = None
            defer_z2 = (b == 0)
            for a in range(NT):
                pool2, tg2 = (psB, "psB") if a % 2 == 0 else (psT, "psT")
                pb = pool2.tile([128, L], F32, name="psB2", tag=tg2)
                inst = nc.tensor.matmul(pb[:], onesb[0:1, :], ncmr[0:1, :],
                                        start=True, stop=False)
                k = gate(tg2, 2, inst)
                # undo the m2 sentinel for padded-j columns
                nc.tensor.matmul(pb[:], onesb[0:1, :], m2rowneg_b,
                                 start=False, stop=True)
                # pb += e_sb (m1col sentinel rides along; the true -1e30
                # bias below dominates it on masked i rows)
                nc.vector.tensor_add(pb[:], pb[:], e_sb[a][:])
                nc.scalar.activation(
                    p21T[a][:], pb[:], Exp,
                    bias=m1c[:, NT * b + a : NT * b + a + 1])
                set_gate(tg2, k, touch(p21T[a]))
                if not defer_z2:
                    for c in range(NT):
                        inst = nc.tensor.matmul(
                            z2ps[:, c : c + 1],
                            p21T[a][:, 128 * c : 128 * (c + 1)],
                            onescol[:], start=(a == 0 and c == 0),
                            stop=(a == NT - 1 and c == NT - 1))
                        if a == 0 and c == 0:
                            kz2 = gate("psS", 1, inst)

            if b == 0:
                # hoist batch 1's e-phase into batch 0's p21/stage-2 window
                # (batch 0 has no earlier work to hide those latency chains);
                # the z2 matmuls are deferred past it so PE isn't queued
                # behind the p21 exp chain
                heads[1] = emit_head(1, nxt[0], nxt[1])
                for a in range(NT):
                    for c in range(NT):
                        inst = nc.tensor.matmul(
                            z2ps[:, c : c + 1],
                            p21T[a][:, 128 * c : 128 * (c + 1)],
                            onescol[:], start=(a == 0 and c == 0),
                            stop=(a == NT - 1 and c == NT - 1))
                        if a == 0 and c == 0:
                            kz2 = gate("psS", 1, inst)

            rz1 = st.tile([128, NT], F32, name="rz1", tag="rz1")
            nc.vector.reciprocal(rz1[:], z1ps[:])
            set_gate("psS", kz1, touch(rz1))

            rz2 = st.tile([128, NT], F32, name="rz2", tag="rz2")
            nc.vector.reciprocal(rz2[:], z2ps[:])
            set_gate("psS", kz2, touch(rz2))

            # stage-2 value operands (loaded early, only now needed by PE)
            touch(xb1t)
            touch(xb2t)

            # ---- stage 2 + enhance + output ----
            for oi, (pT, xval, xnat, rz, y) in enumerate((
                (p12T, xb2t, xb1t, rz1, y1),
                (p21T, xb1t, xb2t, rz2, y2),
            )):
                for a in range(NT):
                    ys = yp.tile([128, 3 * H], BF16, name="ys", tag="ys")
                    for n in range(2):
                        # alternate between the psB and psT rings (psT is
                        # idle during stage 2) so PE can run four groups
                        # ahead of the Act normalizes
                        gid = oi * 2 * NT + 2 * a + n
                        pool, tg = (psB, "psB") if gid % 2 == 0 else (psT, "psT")
                        pt = pool.tile([128, 512], F32, name="psB", tag=tg)
                        k = None
                        for c in range(NT):
                            inst = nc.tensor.matmul(
                                pt[:],
                                pT[c][:, 128 * a : 128 * (a + 1)],
                                xval[:, H * c + 512 * n : H * c + 512 * (n + 1)],
                                start=(c == 0),
                                stop=(c == NT - 1),
                            )
                            if c == 0:
                                k = gate(tg, 2, inst)
                        nc.scalar.activation(
                            ys[:, 512 * n : 512 * (n + 1)], pt[:], Copy,
                            scale=rz[:, a : a + 1])
                        set_gate(tg, k, touch(ys[:, 512 * n : 512 * (n + 1)]))
                    xn = xnat[:, H * a : H * (a + 1)]
                    nc.vector.tensor_sub(ys[:, H : 2 * H], xn, ys[:, 0:H])
                    nc.vector.tensor_mul(ys[:, 2 * H : 3 * H], xn, ys[:, 0:H])
                    rows = slice(128 * a, 128 * (a + 1))
                    if b == BPC - 1 and oi == 1 and a == NT - 1:
                        # last tile: split across all queues to cut the tail
                        for qq in range(3):
                            dmaqs[qq].dma_start(
                                y[b, rows, qq * H : (qq + 1) * H],
                                ys[:, qq * H : (qq + 1) * H])
                    else:
                        # outputs only on SP and Pool; the Act queue must
                        # stay free for Exp/normalize
                        qi = [0, 2, 0, 2, 2, 0, 2, 0][oi * NT + a]
                        dmaqs[qi].dma_start(y[b, rows, :], ys[:])
    if not nc.is_finalized():
        nc.finalize()
    return nc


def kernel(x1_bar, seq_lengths1, x2_bar, seq_lengths2):
    x1_bar = np.ascontiguousarray(x1_bar, dtype=np.float32)
    x2_bar = np.ascontiguousarray(x2_bar, dtype=np.float32)
    sl1 = np.asarray(seq_lengths1).astype(np.int32)
    sl2 = np.asarray(seq_lengths2).astype(np.int32)

    xb1 = x1_bar.astype(NPBF16)
    xb2 = x2_bar.astype(NPBF16)
    xt1 = np.ascontiguousarray(x1_bar.transpose(0, 2, 1))
    xt2 = np.ascontiguousarray(x2_bar.transpose(0, 2, 1))

    ar = np.arange(L, dtype=np.int32)
    pad1 = ar[None, :] >= sl1[:, None]  # [B, L] True on padded i
    pad2 = ar[None, :] >= sl2[:, None]
    m2row = np.where(pad2, -SENT, 0.0).astype(NPBF16)
    m2rowneg = np.where(pad2, SENT, 0.0).astype(NPBF16)
    m1rowneg = np.where(pad1, SENT, 0.0).astype(NPBF16)
    # col masks, swizzled to [128, B*NT]: col[p, b*NT+a] = mask[b, a*128+p]
    def swz(m, val):
        out = np.where(m, val, 0.0).astype(np.float32)  # [B, L]
        return np.ascontiguousarray(
            out.reshape(B, NT, 128).transpose(2, 0, 1).reshape(128, B * NT))
    m1col = swz(pad1, NEG)
    m1colsent = swz(pad1, -SENT)

    if "nc" not in _NC_CACHE:
        _NC_CACHE["nc"] = build_nc()
    nc = _NC_CACHE["nc"]

    in_maps = []
    for c in range(NCORES):
        s = slice(c * BPC, (c + 1) * BPC)
        sc = slice(c * BPC * NT, (c + 1) * BPC * NT)
        in_maps.append({
            "xb1": xb1[s], "xb2": xb2[s], "xt1": xt1[s], "xt2": xt2[s],
            "m2row": m2row[s], "m2rowneg": m2rowneg[s], "m1rowneg": m1rowneg[s],
            "m1col": m1col[:, sc], "m1colsent": m1colsent[:, sc],
        })

    res = run_bass_kernel_spmd(nc, in_maps, core_ids=list(range(NCORES)))
    yd1 = np.concatenate([r["y1"] for r in res.results], axis=0)
    yd2 = np.concatenate([r["y2"] for r in res.results], axis=0)

    y1 = np.empty((B, L, 4 * H), dtype=np.float32)
    y2 = np.empty((B, L, 4 * H), dtype=np.float32)
    y1[:, :, 0:H] = x1_bar
    y2[:, :, 0:H] = x2_bar
    y1[:, :, H:] = yd1.astype(np.float32)
    y2[:, :, H:] = yd2.astype(np.float32)
    return y1, y2
